# revision 6
# baseline (speedup 1.0000x reference)
"""Trainium2 Bass kernel for nn_AttentionBlockT (B=8, D=512, T=2048, K=V=64).

Data-parallel over batch B across 8 NeuronCores: each core computes one batch
element's full TxT causal attention block locally.

Per-core computation (x = minibatch[b], shape (D, T), i.e. x^T in math terms):
  out[0:512, :]   = x                      (pure copy)
  out[512:576, :] = read^T where
      read = softmax((xt @ Wq^T + bq)(xt @ Wk^T + bk)^T / 8 + causal) @ (x^T @ Wv^T + bv)
      xt = [x^T, t], t = 1..T appended feature column.
"""

import math

import numpy as np

B, D, T, KH, VH = 8, 512, 2048, 64, 64
DV = D + VH  # 576
NCORES = 8
NT = T // 128  # 16 t-blocks
NDT = D // 128  # 4 d-tiles

_CACHE = {}


def _build_program():
    import concourse.bass as bass
    import concourse.tile as tile
    from concourse import bacc, mybir
    from concourse.bass import ds, ts

    f32 = mybir.dt.float32
    AF = mybir.ActivationFunctionType
    AX = mybir.AxisListType

    nc = bacc.Bacc("TRN2", target_bir_lowering=False, debug=False)

    x_d = nc.dram_tensor("x", [D, T], f32, kind="ExternalInput")
    wqk_d = nc.dram_tensor("wqk", [128, NDT * 128], f32, kind="ExternalInput")
    affqk_d = nc.dram_tensor("affqk", [2, 128], f32, kind="ExternalInput")
    wv_d = nc.dram_tensor("wv", [128, NDT * VH], f32, kind="ExternalInput")
    affv_d = nc.dram_tensor("affv", [1, VH], f32, kind="ExternalInput")
    trow_d = nc.dram_tensor("trow1", [2, T], f32, kind="ExternalInput")
    ones_d = nc.dram_tensor("ones", [1, T], f32, kind="ExternalInput")
    mask_d = nc.dram_tensor("mask128", [128, 128], f32, kind="ExternalInput")
    ident_d = nc.dram_tensor("ident", [128, 128], f32, kind="ExternalInput")
    out_d = nc.dram_tensor("out", [DV, T], f32, kind="ExternalOutput")

    with tile.TileContext(nc) as tc:
        with (
            tc.tile_pool(name="consts", bufs=1) as consts,
            tc.tile_pool(name="big", bufs=1) as big,
            tc.tile_pool(name="ppool", bufs=2) as ppool,
            tc.tile_pool(name="ptpool", bufs=3) as ptpool,
            tc.tile_pool(name="stats", bufs=8) as stats,
            tc.tile_pool(name="ps1", bufs=6, space="PSUM") as ps1,
            tc.tile_pool(name="ps2", bufs=2, space="PSUM") as ps2,
        ):
            # ---- constants into SBUF ----
            wqk_sb = consts.tile([128, NDT, 128], f32)
            nc.sync.dma_start(out=wqk_sb, in_=wqk_d.ap().rearrange("p (a m) -> p a m", a=NDT))
            wv_sb = consts.tile([128, NDT, VH], f32)
            nc.sync.dma_start(out=wv_sb, in_=wv_d.ap().rearrange("p (a m) -> p a m", a=NDT))
            affqk_sb = consts.tile([2, 128], f32)
            nc.sync.dma_start(out=affqk_sb, in_=affqk_d.ap())
            affv_sb = consts.tile([1, VH], f32)
            nc.sync.dma_start(out=affv_sb, in_=affv_d.ap())
            trow_sb = consts.tile([2, T], f32)
            nc.sync.dma_start(out=trow_sb, in_=trow_d.ap())
            ones_sb = consts.tile([1, T], f32)
            nc.sync.dma_start(out=ones_sb, in_=ones_d.ap())
            mask_sb = consts.tile([128, 128], f32)
            nc.sync.dma_start(out=mask_sb, in_=mask_d.ap())
            ident_sb = consts.tile([128, 128], f32)
            nc.sync.dma_start(out=ident_sb, in_=ident_d.ap())

            # ---- big persistent tiles ----
            x_sb = big.tile([128, NDT, T], f32)       # x d-tiles
            qkt_sb = big.tile([128, T], f32)          # rows 0-63 Q^T, 64-127 K^T
            kt0_sb = big.tile([KH, T], f32)           # K^T at base partition 0
            vt_sb = big.tile([VH, T], f32)            # V^T
            v_sv = big.tile([128, NT, VH], f32)       # V in (s, v) layout per s-tile
            readt_sb = big.tile([VH, T], f32)         # unnormalized read^T
            drow_sb = big.tile([1, T], f32)           # denominators as a row
            rr64_sb = big.tile([VH, T], f32)          # 1/denominator broadcast

            x_dram = x_d.ap().rearrange("(a p) t -> p a t", p=128)
            outx_dram = out_d.ap()[0:D, :].rearrange("(a p) t -> p a t", p=128)

            # ---- phase 1: load x, projections, copy x to out ----
            for j in range(4):
                tsl = ts(j, 512)
                nc.sync.dma_start(out=x_sb[:, :, tsl], in_=x_dram[:, :, tsl])

                qkps = ps1.tile([128, 512], f32, tag="b512")
                for dt in range(NDT):
                    nc.tensor.matmul(
                        out=qkps,
                        lhsT=wqk_sb[:, dt, :],
                        rhs=x_sb[:, dt, tsl],
                        start=(dt == 0),
                        stop=False,
                    )
                nc.tensor.matmul(
                    out=qkps, lhsT=affqk_sb, rhs=trow_sb[:, tsl], start=False, stop=True
                )
                nc.scalar.copy(qkt_sb[:, tsl], qkps)
                # K^T copy to base-partition 0 (SBUF->SBUF DMA moves partitions)
                nc.sync.dma_start(out=kt0_sb[:, tsl], in_=qkt_sb[64:128, tsl])

                vps = ps2.tile([VH, 512], f32, tag="small")
                for dt in range(NDT):
                    nc.tensor.matmul(
                        out=vps,
                        lhsT=wv_sb[:, dt, :],
                        rhs=x_sb[:, dt, tsl],
                        start=(dt == 0),
                        stop=False,
                    )
                nc.tensor.matmul(
                    out=vps, lhsT=affv_sb, rhs=ones_sb[:, tsl], start=False, stop=True
                )
                nc.scalar.copy(vt_sb[:, tsl], vps)

                # stream x back out (rows 0..511 of the output are x verbatim)
                nc.sync.dma_start(out=outx_dram[:, :, tsl], in_=x_sb[:, :, tsl])

            # ---- phase 1b: V^T -> V in (s, v) layout via PE transpose ----
            for g in range(2):  # 8 s-subtiles per PSUM bank group
                vtp = ps1.tile([128, 512], f32, tag="b512")
                for q in range(8):
                    sg = g * 8 + q
                    nc.tensor.transpose(
                        out=vtp[:, ds(q * VH, VH)],
                        in_=vt_sb[:, ts(sg, 128)],
                        identity=ident_sb[0:VH, 0:VH],
                    )
                nc.vector.tensor_copy(
                    v_sv[:, g * 8 : (g + 1) * 8, :].rearrange("p a m -> p (a m)"),
                    vtp,
                )

            # ---- phase 2: attention, one 128-row t-block at a time ----
            for i in range(NT):
                tsl = ts(i, 128)
                nfull = (128 * i) // 512
                rem = 128 * (i % 4) + 128
                nbk = nfull + 1

                banks = []
                for j in range(nfull):
                    lps = ps1.tile([128, 512], f32, tag="b512")
                    nc.tensor.matmul(
                        out=lps,
                        lhsT=qkt_sb[0:64, tsl],
                        rhs=kt0_sb[:, ts(j, 512)],
                        start=True,
                        stop=True,
                    )
                    banks.append((lps, 512))
                lpsl = ps1.tile([128, rem], f32, tag="b512")
                nc.tensor.matmul(
                    out=lpsl,
                    lhsT=qkt_sb[0:64, tsl],
                    rhs=kt0_sb[:, ds(512 * nfull, rem)],
                    start=True,
                    stop=True,
                )
                # causal mask on the diagonal 128 columns
                nc.vector.tensor_add(
                    lpsl[:, ds(rem - 128, 128)], lpsl[:, ds(rem - 128, 128)], mask_sb
                )
                banks.append((lpsl, rem))

                maxc = stats.tile([128, nbk], f32, tag="maxc")
                for bi, (bps, w) in enumerate(banks):
                    nc.vector.reduce_max(maxc[:, bi : bi + 1], bps, axis=AX.X)
                m = stats.tile([128, 1], f32, tag="m")
                nc.vector.reduce_max(m, maxc, axis=AX.X)
                negm8 = stats.tile([128, 1], f32, tag="negm8")
                nc.vector.tensor_scalar_mul(negm8, m, -0.125)

                p_sb = ppool.tile([128, T], f32, tag="p")
                dparts = stats.tile([128, nbk], f32, tag="dparts")
                for bi, (bps, w) in enumerate(banks):
                    nc.scalar.activation(
                        p_sb[:, ds(512 * bi, w)],
                        bps,
                        AF.Exp,
                        bias=negm8,
                        scale=0.125,
                        accum_out=dparts[:, bi : bi + 1],
                    )
                dsum = stats.tile([128, 1], f32, tag="dsum")
                nc.vector.reduce_sum(dsum, dparts, axis=AX.X)

                # denominator column -> row (PE transpose), gathered over blocks
                dtp = ps2.tile([1, 128], f32, tag="small")
                nc.tensor.transpose(out=dtp, in_=dsum, identity=ident_sb)
                nc.scalar.copy(drow_sb[0:1, tsl], dtp)

                # P^T via PE transpose (groups of 4 into one PSUM bank)
                nsub = i + 1
                rps = ps2.tile([VH, 128], f32, tag="small")
                for g in range((nsub + 3) // 4):
                    cnt = min(4, nsub - g * 4)
                    ptp = ps1.tile([128, 512], f32, tag="b512")
                    for q in range(cnt):
                        sg = g * 4 + q
                        nc.tensor.transpose(
                            out=ptp[:, ds(q * 128, 128)],
                            in_=p_sb[:, ts(sg, 128)],
                            identity=ident_sb,
                        )
                    pt_sb = ptpool.tile([128, 512], f32, tag="pt")
                    if g % 2 == 0:
                        nc.scalar.copy(pt_sb[:, 0 : cnt * 128], ptp[:, 0 : cnt * 128])
                    else:
                        nc.vector.tensor_copy(
                            pt_sb[:, 0 : cnt * 128], ptp[:, 0 : cnt * 128]
                        )
                    for q in range(cnt):
                        sg = g * 4 + q
                        nc.tensor.matmul(
                            out=rps,
                            lhsT=v_sv[:, sg, :],
                            rhs=pt_sb[:, ds(q * 128, 128)],
                            start=(sg == 0),
                            stop=(sg == nsub - 1),
                        )
                nc.scalar.copy(readt_sb[:, tsl], rps)

            # ---- phase 3: normalize and write out ----
            rrow = stats.tile([1, T], f32, tag="rrow")
            nc.vector.reciprocal(rrow, drow_sb)
            nc.gpsimd.partition_broadcast(rr64_sb, rrow, channels=VH)
            nc.vector.tensor_mul(readt_sb, readt_sb, rr64_sb)
            nc.sync.dma_start(out=out_d.ap()[D:DV, :], in_=readt_sb)

    nc.compile()
    return nc


def _host_constants(Wk, bk, Wq, bq, Wv, bv):
    f32 = np.float32
    Wcat = np.concatenate([Wq[:, :D], Wk[:, :D]], axis=0)  # (128, 512)
    # wqk[dl, dt*128 + m] = Wcat[m, dt*128 + dl]
    wqk = np.ascontiguousarray(
        np.transpose(Wcat.T.reshape(NDT, 128, 128), (1, 0, 2)).reshape(128, NDT * 128)
    ).astype(f32)
    affqk = np.stack(
        [np.concatenate([Wq[:, D], Wk[:, D]]), np.concatenate([bq, bk])]
    ).astype(f32)
    wv = np.ascontiguousarray(
        np.transpose(Wv.T.reshape(NDT, 128, VH), (1, 0, 2)).reshape(128, NDT * VH)
    ).astype(f32)
    affv = bv.reshape(1, VH).astype(f32)
    trow1 = np.stack(
        [np.arange(1, T + 1, dtype=f32), np.ones(T, dtype=f32)]
    ).astype(f32)
    ones = np.ones((1, T), dtype=f32)
    tt = np.arange(128)
    mask128 = np.where(tt[None, :] <= tt[:, None], 0.0, -1e30).astype(f32)
    ident = np.eye(128, dtype=f32)
    return {
        "wqk": wqk,
        "affqk": affqk,
        "wv": wv,
        "affv": affv,
        "trow1": trow1,
        "ones": ones,
        "mask128": mask128,
        "ident": ident,
    }


def _get_nc():
    if "nc" not in _CACHE:
        _CACHE["nc"] = _build_program()
    return _CACHE["nc"]


def kernel(minibatch, Wk, bk, Wq, bq, Wv, bv):
    from concourse.bass_utils import run_bass_kernel_spmd

    minibatch = np.asarray(minibatch, dtype=np.float32)
    consts = _host_constants(
        np.asarray(Wk, np.float32),
        np.asarray(bk, np.float32),
        np.asarray(Wq, np.float32),
        np.asarray(bq, np.float32),
        np.asarray(Wv, np.float32),
        np.asarray(bv, np.float32),
    )
    nc = _get_nc()
    in_maps = [
        {**consts, "x": np.ascontiguousarray(minibatch[b])} for b in range(NCORES)
    ]
    res = run_bass_kernel_spmd(nc, in_maps, core_ids=list(range(NCORES)))
    return np.stack([res.results[c]["out"] for c in range(NCORES)])
